# revision 50
# baseline (speedup 1.0000x reference)
"""AttentionBlock (GroupNorm32 + self/cross attention + proj + residual) on 8 TRN2 cores.

Sharding: data-parallel over batch. B=8 samples, one per NeuronCore.

v2 design (vs baseline):
  - fp8(e4m3) DoubleRow matmuls for qkv/ekv projections and PV (pairs of
    s-tiles contract 256 rows per MM). Weights pre-scaled x16 on host to
    avoid fp8 subnormals; un-scaled in the PSUM evacuation.
  - v is produced already TRANSPOSED on the PE (vT = nrm^T @ WvT, N=128
    fp8 matmuls) -- no PE transposes, no v/ev evacuation.
  - s-axis tiles: [0:77) encoder tile + 8 aligned self tiles of 128.
  - caption mask folded into vT as data: ones-column = mask (0/1), masked
    ev rows zeroed. No logit bias anywhere; exact vs reference (-1e4 mask
    underflows exp to 0 in fp32 too).
  - exp split: ScalarE real exp -> fp8 for s-tiles 0..4; VectorE
    int8-Schraudolph bit-trick exp -> fp8 for s-tiles 5..8. All wgt tiles
    fp8; PV per (head, tci) = 1 plain MM (enc) + 4 DR MMs.
  - GroupNorm(32) group stats + per-channel broadcast via tiny fp32
    selector matmuls on the PE (no transposes, no DMA broadcast).
  - softmax denominators ride the vT ones-column (PV psum row 64);
    normalize = DVE reciprocal (PSUM read) -> gpsimd partition_broadcast
    -> one [64,1024] TT multiply per head.
  - output projection bf16 from att_all; residual+bias via one STT per
    block; per-block output DMA.
"""

import sys
from contextlib import ExitStack

import numpy as np

for _p in ("/opt/trn_rl_repo",):
    if _p not in sys.path:
        sys.path.insert(0, _p)

import ml_dtypes  # noqa: E402

import concourse.bass as bass  # noqa: E402
import concourse.tile as tile  # noqa: E402
from concourse import bacc, mybir  # noqa: E402
from concourse.masks import make_identity  # noqa: E402

F32 = mybir.dt.float32
BF16 = mybir.dt.bfloat16
FP8 = mybir.dt.float8e4
I8 = mybir.dt.int8
AF = mybir.ActivationFunctionType
ALU = mybir.AluOpType
DRMODE = mybir.MatmulPerfMode.DoubleRow

B, C, HH, WW = 8, 512, 32, 32
T = HH * WW          # 1024
HEADS, CH, S = 8, 64, 77
ST = S + T           # 1101
NT = 9               # s-tiles: [0:77) enc + 8 x 128 self
GCH = 16             # channels per GroupNorm group
N_CORES = 8
WSCALE = 16.0        # host-side weight pre-scale (fp8 subnormal dodge)

# Schraudolph fp8(e4m3) exp: y ~= bitcast8(int8(A8*x + B8))
A8 = 8.0 / float(np.log(2.0))    # 11.5416
B8 = 56.0 - 0.397                # bias 7*8 with mid-point correction

e4m3 = ml_dtypes.float8_e4m3
bf16 = ml_dtypes.bfloat16

DEBUG_DUMPS = False


def tile_rows(j):
    return S if j == 0 else 128


def tile_ssl(j):
    return slice(0, S) if j == 0 else slice(S + 128 * (j - 1), S + 128 * j)


def exp_on_act(j, hh):
    """Per (j, hh): head 0 on ScalarE (real exp), head 1 mostly on VectorE
    (int8-Schraudolph), so both heads' exps run in parallel per s-tile.
    j=0 (enc, both heads) on ScalarE for balance."""
    return hh == 0 or j == 0


def build_program():
    nc = bacc.Bacc("TRN2", target_bir_lowering=False, debug=False)

    x_d = nc.dram_tensor("x", [128, 4, T], F32, kind="ExternalInput")
    enc_d = nc.dram_tensor("enc", [128, 4, 80], FP8, kind="ExternalInput")
    wq_d = nc.dram_tensor("wq", [128, 4, 3 * C], FP8, kind="ExternalInput")
    we_d = nc.dram_tensor("we", [128, 4, 2 * C], FP8, kind="ExternalInput")
    wp_d = nc.dram_tensor("wp", [128, 4, C], BF16, kind="ExternalInput")
    qb_d = nc.dram_tensor("qb", [128, 4, 2], F32, kind="ExternalInput")
    eb_d = nc.dram_tensor("eb", [128, 1], F32, kind="ExternalInput")
    pb_d = nc.dram_tensor("pb", [128, 4], F32, kind="ExternalInput")
    gam_d = nc.dram_tensor("gam", [128, 4], F32, kind="ExternalInput")
    bet_d = nc.dram_tensor("bet", [128, 4], F32, kind="ExternalInput")
    msk_d = nc.dram_tensor("msk", [128, 2], F32, kind="ExternalInput")
    selA_d = nc.dram_tensor("selA", [128, 8], F32, kind="ExternalInput")
    selAT_d = nc.dram_tensor("selAT", [8, 128], F32, kind="ExternalInput")
    out_d = nc.dram_tensor("out", [128, 4, T], F32, kind="ExternalOutput")
    if DEBUG_DUMPS:
        dbg_nrm_d = nc.dram_tensor("dbg_nrm", [128, 4, T], FP8, kind="ExternalOutput")
        dbg_qq_d = nc.dram_tensor("dbg_qq", [128, T], BF16, kind="ExternalOutput")
        dbg_kk_d = nc.dram_tensor("dbg_kk", [128, ST], BF16, kind="ExternalOutput")
        dbg_vt_d = nc.dram_tensor("dbg_vt", [128, NT, 2, 128], FP8, kind="ExternalOutput")
        dbg_we_d = nc.dram_tensor("dbg_we", [128, T], FP8, kind="ExternalOutput")
        dbg_wd_d = nc.dram_tensor("dbg_wd", [128, 4, 2, T], FP8, kind="ExternalOutput")
        dbg_att_d = nc.dram_tensor("dbg_att", [128, 4, T], BF16, kind="ExternalOutput")
        dbg_den_d = nc.dram_tensor("dbg_den", [1, 2, 2, 512], F32, kind="ExternalOutput")
        dbg_rec_d = nc.dram_tensor("dbg_rec", [1, 2, 512], F32, kind="ExternalOutput")
        dbg_rcb_d = nc.dram_tensor("dbg_rcb", [64, T], F32, kind="ExternalOutput")
        dbg_pvn_d = nc.dram_tensor("dbg_pvn", [8, 2, 512], F32, kind="ExternalOutput")

    with tile.TileContext(nc) as tc, ExitStack() as ctx:
        consts = ctx.enter_context(tc.tile_pool(name="consts", bufs=1))
        stats = ctx.enter_context(tc.tile_pool(name="stats", bufs=1))
        kkp = ctx.enter_context(tc.tile_pool(name="kkp", bufs=1))
        hp = ctx.enter_context(tc.tile_pool(name="hp", bufs=2))
        wgtp = ctx.enter_context(tc.tile_pool(name="wgtp", bufs=2))
        psum = ctx.enter_context(tc.tile_pool(name="psum", bufs=2, space="PSUM"))

        # ---- constant loads: x first (critical path: stats -> nrm), split
        # across all three DMA-capable engines; small consts + weights after.
        x_sb = consts.tile([128, 4, T], F32)
        dma_engs = [nc.sync, nc.scalar]
        for i in range(4):
            for hf in range(2):
                eng = dma_engs[(2 * i + hf) % 2]
                eng.dma_start(
                    out=x_sb[:, i, 512 * hf:512 * (hf + 1)],
                    in_=x_d.ap()[:, i, 512 * hf:512 * (hf + 1)])
        enc_sb = consts.tile([128, 4, 80], FP8)
        nc.sync.dma_start(out=enc_sb, in_=enc_d.ap())
        we = consts.tile([128, 4, 2 * C], FP8)
        nc.sync.dma_start(out=we, in_=we_d.ap())
        msk = consts.tile([128, 2], F32)
        nc.sync.dma_start(out=msk, in_=msk_d.ap())
        selA = consts.tile([128, 8], F32)
        nc.sync.dma_start(out=selA, in_=selA_d.ap())
        selAT = consts.tile([8, 128], F32)
        nc.sync.dma_start(out=selAT, in_=selAT_d.ap())
        qb = consts.tile([128, 4, 2], F32)
        nc.sync.dma_start(out=qb, in_=qb_d.ap())
        eb = consts.tile([128, 1], F32)
        nc.sync.dma_start(out=eb, in_=eb_d.ap())
        pb = consts.tile([128, 4], F32)
        nc.sync.dma_start(out=pb, in_=pb_d.ap())
        gam = consts.tile([128, 4], F32)
        nc.sync.dma_start(out=gam, in_=gam_d.ap())
        bet = consts.tile([128, 4], F32)
        nc.sync.dma_start(out=bet, in_=bet_d.ap())
        wq = consts.tile([128, 4, 3 * C], FP8)
        nc.scalar.dma_start(out=wq, in_=wq_d.ap())
        wp = consts.tile([128, 4, C], BF16)
        nc.sync.dma_start(out=wp, in_=wp_d.ap())
        identf = consts.tile([128, 128], F32)
        make_identity(nc, identf)

        # ---- per-pair tensors ----------------------------------------------
        # kk/vT for all 4 pairs at once (ek/evT can run before x arrives).
        kk = [kkp.tile([128, ST], BF16, name=f"kk_{p}") for p in range(4)]
        vT = [kkp.tile([128, NT, 2, 128], FP8, name=f"vT_{p}") for p in range(4)]
        qq = {}
        wenc = {}
        wdr = {}

        def alloc_pair(p):
            qq[p] = hp.tile([128, T], BF16, tag="qq", name=f"qq_{p}")
            for hh in range(2):
                wenc[(p, hh)] = wgtp.tile(
                    [128, T], FP8, tag=f"wenc{hh}", name=f"wenc_{p}_{hh}")
                for pi in range(4):
                    wdr[(p, hh, pi)] = wgtp.tile(
                        [128, 2, T], FP8, tag=f"wdr{hh}{pi}",
                        name=f"wdr_{p}_{hh}_{pi}")

        # ---- early: ek + evT for all pairs (needs only enc/we) ------------
        # ones/mask columns of vT (col 64 of each hh block)
        for p in range(4):
            nc.gpsimd.memset(vT[p][:, :, :, 0:1], 1.0)
            for hh in range(2):
                nc.vector.tensor_copy(
                    out=vT[p][0:S, 0, hh, 0:1], in_=msk[0:S, 0:1])

        for p in range(4):
            # ek: kk[:, 0:77] = (WekT.T @ enc)/16 + ebias
            ek_ps = psum.tile([128, 80], F32, tag="u", bufs=2, name=f"ek_{p}")
            for kp in range(2):
                nc.tensor.matmul(
                    ek_ps,
                    we[:, 2 * kp:2 * kp + 2, 256 * p:256 * p + 128],
                    enc_sb[:, 2 * kp:2 * kp + 2, :],
                    start=(kp == 0), stop=(kp == 1), perf_mode=DRMODE)
            nc.scalar.activation(
                out=kk[p][:, 0:S], in_=ek_ps[:, 0:S], func=AF.Identity,
                bias=eb, scale=1.0 / WSCALE)
            # evT: vT[0:77, 0, :, 0:64] = (enc^T @ WevT) * mask/16
            ev_ps = psum.tile([80, 128], F32, tag="u", bufs=2, name=f"ev_{p}")
            for kp in range(2):
                nc.tensor.matmul(
                    ev_ps[0:S, :],
                    enc_sb[:, 2 * kp:2 * kp + 2, 0:S],
                    we[:, 2 * kp:2 * kp + 2, 256 * p + 128:256 * p + 256],
                    start=(kp == 0), stop=(kp == 1), perf_mode=DRMODE)
            nc.scalar.activation(
                out=vT[p][0:S, 0, :, 64:128],
                in_=ev_ps[0:S, :].rearrange("p (h c) -> p h c", c=64),
                func=AF.Copy, scale=msk[0:S, 1:2])

        # ---- GroupNorm(32) via selector matmuls ----------------------------
        mv = stats.tile([128, 4, 2], F32)
        for i in range(4):
            bnst = stats.tile([128, 2, 6], F32, tag="bnst", bufs=2)
            nc.vector.bn_stats(out=bnst[:, 0, :], in_=x_sb[:, i, 0:512])
            nc.vector.bn_stats(out=bnst[:, 1, :], in_=x_sb[:, i, 512:1024])
            nc.vector.bn_aggr(out=mv[:, i, :], in_=bnst)

        stmx = stats.tile([128, 8], F32)
        nc.vector.tensor_copy(out=stmx[:, 0:4], in_=mv[:, :, 0])
        nc.vector.tensor_mul(out=stmx[:, 4:8], in0=mv[:, :, 0], in1=mv[:, :, 0])
        nc.vector.tensor_add(out=stmx[:, 4:8], in0=stmx[:, 4:8], in1=mv[:, :, 1])

        gps = psum.tile([8, 8], F32, tag="u", bufs=2, name="gps")
        nc.tensor.matmul(gps, selA, stmx, start=True, stop=True)
        mug = stats.tile([8, 4], F32)
        nc.vector.tensor_scalar_mul(out=mug, in0=gps[:, 0:4], scalar1=1.0 / GCH)
        varg = stats.tile([8, 4], F32)
        nc.vector.tensor_scalar_mul(out=varg, in0=gps[:, 4:8], scalar1=1.0 / GCH)
        tmp = stats.tile([8, 4], F32)
        nc.vector.tensor_mul(out=tmp, in0=mug, in1=mug)
        nc.vector.tensor_sub(out=varg, in0=varg, in1=tmp)
        eps_t = stats.tile([8, 1], F32)
        nc.vector.memset(eps_t, 1e-5)
        lnv = stats.tile([8, 4], F32)
        nc.scalar.activation(out=lnv, in_=varg, func=AF.Ln, bias=eps_t, scale=1.0)
        mr = stats.tile([8, 8], F32)
        nc.vector.tensor_copy(out=mr[:, 0:4], in_=mug)
        nc.scalar.activation(out=mr[:, 4:8], in_=lnv, func=AF.Exp, scale=-0.5)

        cols = psum.tile([128, 8], F32, tag="u", bufs=2, name="cols")
        nc.tensor.matmul(cols, selAT, mr, start=True, stop=True)
        a_col = stats.tile([128, 4], F32)
        nc.vector.tensor_mul(out=a_col, in0=cols[:, 4:8], in1=gam)
        b_col = stats.tile([128, 4], F32)
        nc.vector.tensor_mul(out=b_col, in0=cols[:, 0:4], in1=a_col)
        nc.vector.tensor_sub(out=b_col, in0=bet, in1=b_col)

        nrm = consts.tile([128, 4, T], FP8)
        for i in range(4):
            nc.scalar.activation(
                out=nrm[:, i, :], in_=x_sb[:, i, :], func=AF.Identity,
                bias=b_col[:, i:i + 1], scale=a_col[:, i:i + 1])

        att_all = consts.tile([128, 4, T], BF16)

        # ---- per-pair projection units -------------------------------------
        def emit_qk_unit(p, bi, tci):  # noqa: renamed args used via emit_unit
            """q (bi=0) / k (bi=1) projection, t-half tci: 2 DR matmuls."""
            tsl = slice(512 * tci, 512 * (tci + 1))
            mm_ps = psum.tile([128, 512], F32, tag="u", bufs=2,
                              name=f"mm_{p}_{bi}_{tci}")
            cl = 384 * p + 128 * bi
            for kp in range(2):
                nc.tensor.matmul(
                    mm_ps,
                    wq[:, 2 * kp:2 * kp + 2, cl:cl + 128],
                    nrm[:, 2 * kp:2 * kp + 2, tsl],
                    start=(kp == 0), stop=(kp == 1), perf_mode=DRMODE)
            dest = qq[p][:, tsl] if bi == 0 else kk[p][:, S + 512 * tci:S + 512 * (tci + 1)]
            nc.scalar.activation(
                out=dest, in_=mm_ps, func=AF.Identity,
                bias=qb[:, p, bi:bi + 1], scale=1.0 / WSCALE)

        def emit_vt_unit(p, j):  # noqa
            """self vT tile j (1..8): vT[:, j, :, 0:64] = nrm^T @ WvT / 16."""
            tsl = slice(128 * (j - 1), 128 * j)
            vt_ps = psum.tile([128, 128], F32, tag="u", bufs=2,
                              name=f"vt_{p}_{j}")
            cl = 384 * p + 256
            for kc in range(4):
                nc.tensor.matmul(
                    vt_ps, nrm[:, kc, tsl], wq[:, kc, cl:cl + 128],
                    start=(kc == 0), stop=(kc == 3))
            nc.vector.tensor_scalar(
                out=vT[p][:, j, :, 64:128],
                in0=vt_ps.rearrange("p (h c) -> p h c", c=64),
                scalar1=1.0 / WSCALE, scalar2=0.0,
                op0=ALU.mult, op1=ALU.add)

        def emit_unit(u):
            if u[0] == "qk":
                emit_qk_unit(u[1], u[2], u[3])
            else:
                emit_vt_unit(u[1], u[2])

        # ---- attention -----------------------------------------------------
        def emit_qk_exp(p, j):
            rows = tile_rows(j)
            ssl = tile_ssl(j)
            # per-(hh, tci) [128, 512] psum tiles: fine-grained ring so the
            # exp of tile (j, h0, t0) overlaps QK of (j, ., t1) and frees its
            # slot early for j+1.
            qk = {}
            for tci in range(2):
                for hh in range(2):
                    qk[(hh, tci)] = psum.tile(
                        [128, 512], F32, tag="qk", bufs=4,
                        name=f"qk_{p}_{j}_{hh}_{tci}")
                    rlo = 64 * hh
                    nc.tensor.matmul(
                        qk[(hh, tci)][0:rows, :],
                        kk[p][rlo:rlo + 64, ssl],
                        qq[p][rlo:rlo + 64, 512 * tci:512 * (tci + 1)],
                    )
                # emit exps for this tci right away (tci-granular drain)
                for hh in range(2):
                    if j == 0:
                        dsl = wenc[(p, hh)][0:rows, 512 * tci:512 * (tci + 1)]
                        dsl8 = None
                    else:
                        pi = (j - 1) // 2
                        sl = (j - 1) % 2
                        dsl = wdr[(p, hh, pi)][0:rows, sl,
                                               512 * tci:512 * (tci + 1)]
                        dsl8 = wdr[(p, hh, pi)].bitcast(I8)[
                            0:rows, sl, 512 * tci:512 * (tci + 1)]
                    if exp_on_act(j, hh):
                        nc.scalar.activation(
                            out=dsl, in_=qk[(hh, tci)][0:rows, :],
                            func=AF.Exp, scale=0.125)
                    else:
                        nc.vector.tensor_scalar(
                            out=dsl8, in0=qk[(hh, tci)][0:rows, :],
                            scalar1=A8 * 0.125, scalar2=B8,
                            op0=ALU.mult, op1=ALU.add)

        def emit_pv_stage(p, pvt, hh, stage):
            """stage 0: encoder tile (start); 1..4: DR pair pi=stage-1 (stop
            on 4). tci-inner so matmuls sharing a stationary operand are
            adjacent."""
            for tci in range(2):
                tsl = slice(512 * tci, 512 * (tci + 1))
                if stage == 0:
                    nc.tensor.matmul(
                        pvt[tci],
                        vT[p][0:S, 0, hh, 0:128],
                        wenc[(p, hh)][0:S, tsl],
                        start=True, stop=False, skip_group_check=True)
                else:
                    pi = stage - 1
                    nc.tensor.matmul(
                        pvt[tci],
                        vT[p][:, 2 * pi + 1:2 * pi + 3, hh, 0:128],
                        wdr[(p, hh, pi)][:, :, tsl],
                        start=False, stop=(pi == 3), perf_mode=DRMODE,
                        skip_group_check=True)

        def emit_normalize_hh(p, pvt, hh):
            recipb_h = hp.tile([64, T], F32, tag=f"recipb{hh}",
                               name=f"recipb_{p}_{hh}")
            for tci in range(2):
                # den rides psum row 0 (ones-column is vT col 0), so the
                # custom-DVE recip -- which always reads partition 0 -- can
                # take the PSUM AP directly; no ScalarE copy.
                rec_g = hp.tile([1, 512], F32, tag=f"rec{hh}{tci}",
                                name=f"rec_{p}_{hh}_{tci}")
                nc.vector.reciprocal_approx_fast(
                    out=rec_g, in_=pvt[tci][0:1, :])
                nc.gpsimd.partition_broadcast(
                    recipb_h[:, 512 * tci:512 * (tci + 1)], rec_g)
            if DEBUG_DUMPS and p == 0 and hh == 0:
                nc.sync.dma_start(out=dbg_rec_d.ap(), in_=rec_h)
                nc.sync.dma_start(out=dbg_rcb_d.ap(), in_=recipb_h)
                pvn = stats.tile([8, 2, 512], F32, name="pvn_sb")
                for tci in range(2):
                    nc.vector.tensor_copy(out=pvn[:, tci, :],
                                          in_=pvt[tci][0:8, :])
                nc.sync.dma_start(out=dbg_pvn_d.ap(), in_=pvn)
            rlo = 64 * hh
            for tci in range(2):
                nc.vector.tensor_mul(
                    out=att_all[rlo:rlo + 64, p, 512 * tci:512 * (tci + 1)],
                    in0=pvt[tci][64:128, :],
                    in1=recipb_h[:, 512 * tci:512 * (tci + 1)])

        # prologue: pair 0 q/k projections only (vT units interleave into
        # pair 0's own j-loop; PV needs them only at pair end)
        alloc_pair(0)
        for bi in range(2):
            for tci in range(2):
                emit_qk_unit(0, bi, tci)

        for p in range(4):
            units = [("vt", p, j) for j in range(1, NT)]
            if p < 3:
                alloc_pair(p + 1)
                units += [("qk", p + 1, bi, tci)
                          for bi in range(2) for tci in range(2)]
            for j in range(NT):
                emit_qk_exp(p, j)
                if units:
                    take = 1 if j < 4 else 2
                    for u in units[:take]:
                        emit_unit(u)
                    units = units[take:]
            for hh in range(2):
                pvt = [psum.tile([128, 512], F32, tag="pv", bufs=2,
                                 name=f"pv_{p}_{hh}_{tci}") for tci in range(2)]
                for stage in range(5):
                    emit_pv_stage(p, pvt, hh, stage)
                emit_normalize_hh(p, pvt, hh)
                if hh == 0:
                    for u in units:
                        emit_unit(u)
                    units = []
            if DEBUG_DUMPS and p == 0:
                nc.sync.dma_start(out=dbg_qq_d.ap(), in_=qq[0])
                nc.sync.dma_start(out=dbg_we_d.ap(), in_=wenc[(0, 0)])
                for pi in range(4):
                    nc.sync.dma_start(out=dbg_wd_d.ap()[:, pi, :, :],
                                      in_=wdr[(0, 0, pi)])

        if DEBUG_DUMPS:
            nc.sync.dma_start(out=dbg_nrm_d.ap(), in_=nrm)
            nc.sync.dma_start(out=dbg_kk_d.ap(), in_=kk[0])
            nc.sync.dma_start(out=dbg_vt_d.ap(), in_=vT[0])
            nc.sync.dma_start(out=dbg_att_d.ap(), in_=att_all)

        # ---- output projection + residual ----------------------------------
        # residual x is added into the proj psum via an identity matmul (PE
        # and ScalarE are idle at the tail; VectorE is still draining exps).
        opool = ctx.enter_context(tc.tile_pool(name="opool", bufs=2))
        for i in range(4):
            for tci in range(2):
                tsl = slice(512 * tci, 512 * (tci + 1))
                pr_ps = psum.tile([128, 512], F32, tag="u", bufs=2,
                                  name=f"pr_{i}_{tci}")
                nc.tensor.matmul(
                    pr_ps, identf, x_sb[:, i, tsl],
                    start=True, stop=False, skip_group_check=True)
                for kc in range(4):
                    nc.tensor.matmul(
                        pr_ps, wp[:, kc, 128 * i:128 * (i + 1)],
                        att_all[:, kc, tsl],
                        start=False, stop=(kc == 3), skip_group_check=True,
                    )
                out_sb = opool.tile([128, 512], F32, tag="osb")
                nc.scalar.activation(
                    out=out_sb, in_=pr_ps, func=AF.Identity,
                    bias=pb[:, i:i + 1], scale=1.0)
                eng = nc.sync if tci == 0 else nc.scalar
                eng.dma_start(out=out_d.ap()[:, i, tsl], in_=out_sb)

    nc.compile()
    return nc


def _to_part_major(a, inner):
    """[C, inner...] with C=512 -> [128, 4, inner] (c = 128*i + p)."""
    return np.ascontiguousarray(
        a.reshape(4, 128, inner).transpose(1, 0, 2))


def prep_inputs(x, encoder_out, capt_attn_mask, norm_scale, norm_bias,
                qkv_w, qkv_b, ekv_w, ekv_b, proj_w, proj_b):
    """Host-side marshalling: shard over batch + transpose/cast weights."""
    x = np.asarray(x, np.float32).reshape(B, C, T)
    enc = np.asarray(encoder_out, np.float32)
    mask = np.asarray(capt_attn_mask).astype(bool)

    x_dev = x.reshape(B, 4, 128, T).transpose(0, 2, 1, 3)
    enc_pad = np.zeros((B, C, 80), np.float32)
    enc_pad[:, :, 0:S] = enc
    enc_dev = enc_pad.reshape(B, 4, 128, 80).transpose(0, 2, 1, 3).astype(e4m3)

    # mask columns: col0 = m (0/1, rows>=77 -> 1), col1 = m/16
    mvec = np.ones((B, 128), np.float32)
    mvec[:, 0:S] = mask.astype(np.float32)
    msk = np.stack([mvec, mvec / WSCALE], axis=2)  # [B, 128, 2]

    # weight rows permuted into per-pair block layout:
    # [q_h|q_h1|k_h|k_h1|v_h|v_h1] per pair.
    qperm = np.array([
        192 * (2 * p + hh) + 64 * b + o
        for p in range(4) for b in range(3) for hh in range(2) for o in range(64)
    ])
    eperm = np.array([
        128 * (2 * p + hh) + 64 * b + o
        for p in range(4) for b in range(2) for hh in range(2) for o in range(64)
    ])
    wq_t = _to_part_major(
        np.asarray(qkv_w, np.float32)[qperm].T * WSCALE, 3 * C).astype(e4m3)
    we_t = _to_part_major(
        np.asarray(ekv_w, np.float32)[eperm].T * WSCALE, 2 * C).astype(e4m3)
    wp_t = _to_part_major(np.asarray(proj_w, np.float32).T, C).astype(bf16)

    qkv_b = np.asarray(qkv_b, np.float32)
    ekv_b = np.asarray(ekv_b, np.float32)
    # v / ev biases are folded nowhere -- must be zero (true for this problem)
    for p in range(4):
        for hh in range(2):
            h = 2 * p + hh
            assert np.all(qkv_b[192 * h + 128:192 * h + 192] == 0.0), "v bias != 0"
            assert np.all(ekv_b[128 * h + 64:128 * h + 128] == 0.0), "ev bias != 0"
    qbm = np.zeros((128, 4, 2), np.float32)
    ebm = np.zeros((128, 1), np.float32)
    for p in range(4):
        h = 2 * p
        for bi in range(2):
            qbm[0:64, p, bi] = qkv_b[192 * h + 64 * bi:192 * h + 64 * bi + 64]
            qbm[64:128, p, bi] = qkv_b[192 * (h + 1) + 64 * bi:192 * (h + 1) + 64 * bi + 64]
        ebm[0:64, 0] = ekv_b[128 * h:128 * h + 64]
        ebm[64:128, 0] = ekv_b[128 * (h + 1):128 * (h + 1) + 64]
    pbm = np.ascontiguousarray(np.asarray(proj_b, np.float32).reshape(4, 128).T)
    gamm = np.ascontiguousarray(np.asarray(norm_scale, np.float32).reshape(4, 128).T)
    betm = np.ascontiguousarray(np.asarray(norm_bias, np.float32).reshape(4, 128).T)

    selA = np.zeros((128, 8), np.float32)
    for pp in range(128):
        selA[pp, pp // 16] = 1.0
    selAT = np.ascontiguousarray(selA.T)

    shared = {"wq": wq_t, "we": we_t, "wp": wp_t, "qb": qbm, "eb": ebm,
              "pb": pbm, "gam": gamm, "bet": betm, "selA": selA,
              "selAT": selAT}
    in_maps = []
    for b in range(B):
        m = dict(shared)
        m["x"] = np.ascontiguousarray(x_dev[b])
        m["enc"] = np.ascontiguousarray(enc_dev[b])
        m["msk"] = np.ascontiguousarray(msk[b])
        in_maps.append(m)
    return in_maps


def gather_output(results):
    out = np.stack([r["out"] for r in results])  # [8, 128, 4, T]
    return np.ascontiguousarray(
        out.transpose(0, 2, 1, 3).reshape(B, C, HH, WW).astype(np.float32))


_NC = None


def _get_nc():
    global _NC
    if _NC is None:
        _NC = build_program()
    return _NC


def kernel(**inputs) -> np.ndarray:
    from concourse.bass_utils import run_bass_kernel_spmd

    nc = _get_nc()
    in_maps = prep_inputs(**inputs)
    res = run_bass_kernel_spmd(nc, in_maps, core_ids=list(range(N_CORES)))
    return gather_output(res.results)


if __name__ == "__main__":
    nc = build_program()
    print("program built ok")


# revision 51
# speedup vs baseline: 1.0766x; 1.0766x over previous
"""AttentionBlock (GroupNorm32 + self/cross attention + proj + residual) on 8 TRN2 cores.

Sharding: data-parallel over batch. B=8 samples, one per NeuronCore.

v2 design (vs baseline):
  - fp8(e4m3) DoubleRow matmuls for qkv/ekv projections and PV (pairs of
    s-tiles contract 256 rows per MM). Weights pre-scaled x16 on host to
    avoid fp8 subnormals; un-scaled in the PSUM evacuation.
  - v is produced already TRANSPOSED on the PE (vT = nrm^T @ WvT, N=128
    fp8 matmuls) -- no PE transposes, no v/ev evacuation.
  - s-axis tiles: [0:77) encoder tile + 8 aligned self tiles of 128.
  - caption mask folded into vT as data: ones-column = mask (0/1), masked
    ev rows zeroed. No logit bias anywhere; exact vs reference (-1e4 mask
    underflows exp to 0 in fp32 too).
  - exp split: ScalarE real exp -> fp8 for s-tiles 0..4; VectorE
    int8-Schraudolph bit-trick exp -> fp8 for s-tiles 5..8. All wgt tiles
    fp8; PV per (head, tci) = 1 plain MM (enc) + 4 DR MMs.
  - GroupNorm(32) group stats + per-channel broadcast via tiny fp32
    selector matmuls on the PE (no transposes, no DMA broadcast).
  - softmax denominators ride the vT ones-column (PV psum row 64);
    normalize = DVE reciprocal (PSUM read) -> gpsimd partition_broadcast
    -> one [64,1024] TT multiply per head.
  - output projection bf16 from att_all; residual+bias via one STT per
    block; per-block output DMA.
"""

import sys
from contextlib import ExitStack

import numpy as np

for _p in ("/opt/trn_rl_repo",):
    if _p not in sys.path:
        sys.path.insert(0, _p)

import ml_dtypes  # noqa: E402

import concourse.bass as bass  # noqa: E402
import concourse.tile as tile  # noqa: E402
from concourse import bacc, mybir  # noqa: E402
from concourse.masks import make_identity  # noqa: E402

F32 = mybir.dt.float32
BF16 = mybir.dt.bfloat16
FP8 = mybir.dt.float8e4
I8 = mybir.dt.int8
AF = mybir.ActivationFunctionType
ALU = mybir.AluOpType
DRMODE = mybir.MatmulPerfMode.DoubleRow

B, C, HH, WW = 8, 512, 32, 32
T = HH * WW          # 1024
HEADS, CH, S = 8, 64, 77
ST = S + T           # 1101
NT = 9               # s-tiles: [0:77) enc + 8 x 128 self
GCH = 16             # channels per GroupNorm group
N_CORES = 8
WSCALE = 16.0        # host-side weight pre-scale (fp8 subnormal dodge)

# Schraudolph fp8(e4m3) exp: y ~= bitcast8(int8(A8*x + B8))
A8 = 8.0 / float(np.log(2.0))    # 11.5416
B8 = 56.0 - 0.397                # bias 7*8 with mid-point correction

e4m3 = ml_dtypes.float8_e4m3
bf16 = ml_dtypes.bfloat16

DEBUG_DUMPS = False


def tile_rows(j):
    return S if j == 0 else 128


def tile_ssl(j):
    return slice(0, S) if j == 0 else slice(S + 128 * (j - 1), S + 128 * j)


def exp_on_act(j, hh):
    """Per (j, hh): head 0 on ScalarE (real exp), head 1 mostly on VectorE
    (int8-Schraudolph), so both heads' exps run in parallel per s-tile.
    j=0 (enc, both heads) and j=1 of head 1 on ScalarE for balance."""
    return hh == 0 or j <= 1


def build_program():
    nc = bacc.Bacc("TRN2", target_bir_lowering=False, debug=False)

    x_d = nc.dram_tensor("x", [128, 4, T], F32, kind="ExternalInput")
    enc_d = nc.dram_tensor("enc", [128, 4, 80], FP8, kind="ExternalInput")
    wq_d = nc.dram_tensor("wq", [128, 4, 3 * C], FP8, kind="ExternalInput")
    we_d = nc.dram_tensor("we", [128, 4, 2 * C], FP8, kind="ExternalInput")
    wp_d = nc.dram_tensor("wp", [128, 4, C], BF16, kind="ExternalInput")
    qb_d = nc.dram_tensor("qb", [128, 4, 2], F32, kind="ExternalInput")
    eb_d = nc.dram_tensor("eb", [128, 1], F32, kind="ExternalInput")
    pb_d = nc.dram_tensor("pb", [128, 4], F32, kind="ExternalInput")
    gam_d = nc.dram_tensor("gam", [128, 4], F32, kind="ExternalInput")
    bet_d = nc.dram_tensor("bet", [128, 4], F32, kind="ExternalInput")
    msk_d = nc.dram_tensor("msk", [128, 2], F32, kind="ExternalInput")
    selA_d = nc.dram_tensor("selA", [128, 8], F32, kind="ExternalInput")
    selAT_d = nc.dram_tensor("selAT", [8, 128], F32, kind="ExternalInput")
    out_d = nc.dram_tensor("out", [128, 4, T], F32, kind="ExternalOutput")
    if DEBUG_DUMPS:
        dbg_nrm_d = nc.dram_tensor("dbg_nrm", [128, 4, T], FP8, kind="ExternalOutput")
        dbg_qq_d = nc.dram_tensor("dbg_qq", [128, T], BF16, kind="ExternalOutput")
        dbg_kk_d = nc.dram_tensor("dbg_kk", [128, ST], BF16, kind="ExternalOutput")
        dbg_vt_d = nc.dram_tensor("dbg_vt", [128, NT, 2, 128], FP8, kind="ExternalOutput")
        dbg_we_d = nc.dram_tensor("dbg_we", [128, T], FP8, kind="ExternalOutput")
        dbg_wd_d = nc.dram_tensor("dbg_wd", [128, 4, 2, T], FP8, kind="ExternalOutput")
        dbg_att_d = nc.dram_tensor("dbg_att", [128, 4, T], BF16, kind="ExternalOutput")
        dbg_den_d = nc.dram_tensor("dbg_den", [1, 2, 2, 512], F32, kind="ExternalOutput")
        dbg_rec_d = nc.dram_tensor("dbg_rec", [1, 2, 512], F32, kind="ExternalOutput")
        dbg_rcb_d = nc.dram_tensor("dbg_rcb", [64, T], F32, kind="ExternalOutput")
        dbg_pvn_d = nc.dram_tensor("dbg_pvn", [8, 2, 512], F32, kind="ExternalOutput")

    with tile.TileContext(nc) as tc, ExitStack() as ctx:
        consts = ctx.enter_context(tc.tile_pool(name="consts", bufs=1))
        stats = ctx.enter_context(tc.tile_pool(name="stats", bufs=1))
        kkp = ctx.enter_context(tc.tile_pool(name="kkp", bufs=1))
        hp = ctx.enter_context(tc.tile_pool(name="hp", bufs=2))
        wgtp = ctx.enter_context(tc.tile_pool(name="wgtp", bufs=2))
        psum = ctx.enter_context(tc.tile_pool(name="psum", bufs=2, space="PSUM"))

        # ---- constant loads (small first; x in 4 chunks) -------------------
        enc_sb = consts.tile([128, 4, 80], FP8)
        nc.sync.dma_start(out=enc_sb, in_=enc_d.ap())
        we = consts.tile([128, 4, 2 * C], FP8)
        nc.sync.dma_start(out=we, in_=we_d.ap())
        msk = consts.tile([128, 2], F32)
        nc.sync.dma_start(out=msk, in_=msk_d.ap())
        selA = consts.tile([128, 8], F32)
        nc.sync.dma_start(out=selA, in_=selA_d.ap())
        selAT = consts.tile([8, 128], F32)
        nc.sync.dma_start(out=selAT, in_=selAT_d.ap())
        qb = consts.tile([128, 4, 2], F32)
        nc.sync.dma_start(out=qb, in_=qb_d.ap())
        eb = consts.tile([128, 1], F32)
        nc.sync.dma_start(out=eb, in_=eb_d.ap())
        pb = consts.tile([128, 4], F32)
        nc.sync.dma_start(out=pb, in_=pb_d.ap())
        gam = consts.tile([128, 4], F32)
        nc.sync.dma_start(out=gam, in_=gam_d.ap())
        bet = consts.tile([128, 4], F32)
        nc.sync.dma_start(out=bet, in_=bet_d.ap())
        x_sb = consts.tile([128, 4, T], F32)
        for i in range(4):
            for hf in range(2):
                eng = nc.sync if (2 * i + hf) % 2 == 0 else nc.scalar
                eng.dma_start(
                    out=x_sb[:, i, 512 * hf:512 * (hf + 1)],
                    in_=x_d.ap()[:, i, 512 * hf:512 * (hf + 1)])
        wq = consts.tile([128, 4, 3 * C], FP8)
        nc.scalar.dma_start(out=wq, in_=wq_d.ap())
        wp = consts.tile([128, 4, C], BF16)
        nc.sync.dma_start(out=wp, in_=wp_d.ap())
        identf = consts.tile([128, 128], F32)
        make_identity(nc, identf)

        # ---- per-pair tensors ----------------------------------------------
        # kk/vT for all 4 pairs at once (ek/evT can run before x arrives).
        kk = [kkp.tile([128, ST], BF16, name=f"kk_{p}") for p in range(4)]
        vT = [kkp.tile([128, NT, 2, 128], FP8, name=f"vT_{p}") for p in range(4)]
        qq = {}
        wenc = {}
        wdr = {}

        def alloc_pair(p):
            qq[p] = hp.tile([128, T], BF16, tag="qq", name=f"qq_{p}")
            for hh in range(2):
                wenc[(p, hh)] = wgtp.tile(
                    [128, T], FP8, tag=f"wenc{hh}", name=f"wenc_{p}_{hh}")
                for pi in range(4):
                    wdr[(p, hh, pi)] = wgtp.tile(
                        [128, 2, T], FP8, tag=f"wdr{hh}{pi}",
                        name=f"wdr_{p}_{hh}_{pi}")

        # ---- early: ek + evT for all pairs (needs only enc/we) ------------
        # ones/mask columns of vT (col 64 of each hh block)
        for p in range(4):
            nc.gpsimd.memset(vT[p][:, :, :, 0:1], 1.0)
            for hh in range(2):
                nc.vector.tensor_copy(
                    out=vT[p][0:S, 0, hh, 0:1], in_=msk[0:S, 0:1])

        for p in range(4):
            # ek: kk[:, 0:77] = (WekT.T @ enc)/16 + ebias
            ek_ps = psum.tile([128, 80], F32, tag="u", bufs=2, name=f"ek_{p}")
            for kp in range(2):
                nc.tensor.matmul(
                    ek_ps,
                    we[:, 2 * kp:2 * kp + 2, 256 * p:256 * p + 128],
                    enc_sb[:, 2 * kp:2 * kp + 2, :],
                    start=(kp == 0), stop=(kp == 1), perf_mode=DRMODE)
            nc.scalar.activation(
                out=kk[p][:, 0:S], in_=ek_ps[:, 0:S], func=AF.Identity,
                bias=eb, scale=1.0 / WSCALE)
            # evT: vT[0:77, 0, :, 0:64] = (enc^T @ WevT) * mask/16
            ev_ps = psum.tile([80, 128], F32, tag="u", bufs=2, name=f"ev_{p}")
            for kp in range(2):
                nc.tensor.matmul(
                    ev_ps[0:S, :],
                    enc_sb[:, 2 * kp:2 * kp + 2, 0:S],
                    we[:, 2 * kp:2 * kp + 2, 256 * p + 128:256 * p + 256],
                    start=(kp == 0), stop=(kp == 1), perf_mode=DRMODE)
            nc.scalar.activation(
                out=vT[p][0:S, 0, :, 64:128],
                in_=ev_ps[0:S, :].rearrange("p (h c) -> p h c", c=64),
                func=AF.Copy, scale=msk[0:S, 1:2])

        # ---- GroupNorm(32) via selector matmuls ----------------------------
        mv = stats.tile([128, 4, 2], F32)
        for i in range(4):
            bnst = stats.tile([128, 2, 6], F32, tag="bnst", bufs=2)
            nc.vector.bn_stats(out=bnst[:, 0, :], in_=x_sb[:, i, 0:512])
            nc.vector.bn_stats(out=bnst[:, 1, :], in_=x_sb[:, i, 512:1024])
            nc.vector.bn_aggr(out=mv[:, i, :], in_=bnst)

        stmx = stats.tile([128, 8], F32)
        nc.vector.tensor_copy(out=stmx[:, 0:4], in_=mv[:, :, 0])
        nc.vector.tensor_mul(out=stmx[:, 4:8], in0=mv[:, :, 0], in1=mv[:, :, 0])
        nc.vector.tensor_add(out=stmx[:, 4:8], in0=stmx[:, 4:8], in1=mv[:, :, 1])

        gps = psum.tile([8, 8], F32, tag="u", bufs=2, name="gps")
        nc.tensor.matmul(gps, selA, stmx, start=True, stop=True)
        mug = stats.tile([8, 4], F32)
        nc.vector.tensor_scalar_mul(out=mug, in0=gps[:, 0:4], scalar1=1.0 / GCH)
        varg = stats.tile([8, 4], F32)
        nc.vector.tensor_scalar_mul(out=varg, in0=gps[:, 4:8], scalar1=1.0 / GCH)
        tmp = stats.tile([8, 4], F32)
        nc.vector.tensor_mul(out=tmp, in0=mug, in1=mug)
        nc.vector.tensor_sub(out=varg, in0=varg, in1=tmp)
        eps_t = stats.tile([8, 1], F32)
        nc.vector.memset(eps_t, 1e-5)
        lnv = stats.tile([8, 4], F32)
        nc.scalar.activation(out=lnv, in_=varg, func=AF.Ln, bias=eps_t, scale=1.0)
        mr = stats.tile([8, 8], F32)
        nc.vector.tensor_copy(out=mr[:, 0:4], in_=mug)
        nc.scalar.activation(out=mr[:, 4:8], in_=lnv, func=AF.Exp, scale=-0.5)

        cols = psum.tile([128, 8], F32, tag="u", bufs=2, name="cols")
        nc.tensor.matmul(cols, selAT, mr, start=True, stop=True)
        a_col = stats.tile([128, 4], F32)
        nc.vector.tensor_mul(out=a_col, in0=cols[:, 4:8], in1=gam)
        b_col = stats.tile([128, 4], F32)
        nc.vector.tensor_mul(out=b_col, in0=cols[:, 0:4], in1=a_col)
        nc.vector.tensor_sub(out=b_col, in0=bet, in1=b_col)

        nrm = consts.tile([128, 4, T], FP8)
        for i in range(4):
            nc.scalar.activation(
                out=nrm[:, i, :], in_=x_sb[:, i, :], func=AF.Identity,
                bias=b_col[:, i:i + 1], scale=a_col[:, i:i + 1])

        att_all = consts.tile([128, 4, T], BF16)

        # ---- per-pair projection units -------------------------------------
        def emit_qk_unit(p, bi, tci):  # noqa: renamed args used via emit_unit
            """q (bi=0) / k (bi=1) projection, t-half tci: 2 DR matmuls."""
            tsl = slice(512 * tci, 512 * (tci + 1))
            mm_ps = psum.tile([128, 512], F32, tag="u", bufs=2,
                              name=f"mm_{p}_{bi}_{tci}")
            cl = 384 * p + 128 * bi
            for kp in range(2):
                nc.tensor.matmul(
                    mm_ps,
                    wq[:, 2 * kp:2 * kp + 2, cl:cl + 128],
                    nrm[:, 2 * kp:2 * kp + 2, tsl],
                    start=(kp == 0), stop=(kp == 1), perf_mode=DRMODE)
            dest = qq[p][:, tsl] if bi == 0 else kk[p][:, S + 512 * tci:S + 512 * (tci + 1)]
            nc.scalar.activation(
                out=dest, in_=mm_ps, func=AF.Identity,
                bias=qb[:, p, bi:bi + 1], scale=1.0 / WSCALE)

        def emit_vt_unit(p, j):  # noqa
            """self vT tile j (1..8): vT[:, j, :, 0:64] = nrm^T @ WvT / 16."""
            tsl = slice(128 * (j - 1), 128 * j)
            vt_ps = psum.tile([128, 128], F32, tag="u", bufs=2,
                              name=f"vt_{p}_{j}")
            cl = 384 * p + 256
            for kc in range(4):
                nc.tensor.matmul(
                    vt_ps, nrm[:, kc, tsl], wq[:, kc, cl:cl + 128],
                    start=(kc == 0), stop=(kc == 3))
            nc.vector.tensor_scalar(
                out=vT[p][:, j, :, 64:128],
                in0=vt_ps.rearrange("p (h c) -> p h c", c=64),
                scalar1=1.0 / WSCALE, scalar2=0.0,
                op0=ALU.mult, op1=ALU.add)

        def emit_unit(u):
            if u[0] == "qk":
                emit_qk_unit(u[1], u[2], u[3])
            else:
                emit_vt_unit(u[1], u[2])

        # ---- attention -----------------------------------------------------
        def emit_qk_exp(p, j):
            rows = tile_rows(j)
            ssl = tile_ssl(j)
            # per-(hh, tci) [128, 512] psum tiles: fine-grained ring so the
            # exp of tile (j, h0, t0) overlaps QK of (j, ., t1) and frees its
            # slot early for j+1.
            qk = {}
            for tci in range(2):
                for hh in range(2):
                    qk[(hh, tci)] = psum.tile(
                        [128, 512], F32, tag="qk", bufs=4,
                        name=f"qk_{p}_{j}_{hh}_{tci}")
                    rlo = 64 * hh
                    nc.tensor.matmul(
                        qk[(hh, tci)][0:rows, :],
                        kk[p][rlo:rlo + 64, ssl],
                        qq[p][rlo:rlo + 64, 512 * tci:512 * (tci + 1)],
                    )
                # emit exps for this tci right away (tci-granular drain)
                for hh in range(2):
                    if j == 0:
                        dsl = wenc[(p, hh)][0:rows, 512 * tci:512 * (tci + 1)]
                        dsl8 = None
                    else:
                        pi = (j - 1) // 2
                        sl = (j - 1) % 2
                        dsl = wdr[(p, hh, pi)][0:rows, sl,
                                               512 * tci:512 * (tci + 1)]
                        dsl8 = wdr[(p, hh, pi)].bitcast(I8)[
                            0:rows, sl, 512 * tci:512 * (tci + 1)]
                    if exp_on_act(j, hh):
                        nc.scalar.activation(
                            out=dsl, in_=qk[(hh, tci)][0:rows, :],
                            func=AF.Exp, scale=0.125)
                    else:
                        nc.vector.tensor_scalar(
                            out=dsl8, in0=qk[(hh, tci)][0:rows, :],
                            scalar1=A8 * 0.125, scalar2=B8,
                            op0=ALU.mult, op1=ALU.add)

        def emit_pv_stage(p, pvt, hh, stage):
            """stage 0: encoder tile (start); 1..4: DR pair pi=stage-1 (stop
            on 4). tci-inner so matmuls sharing a stationary operand are
            adjacent."""
            for tci in range(2):
                tsl = slice(512 * tci, 512 * (tci + 1))
                if stage == 0:
                    nc.tensor.matmul(
                        pvt[tci],
                        vT[p][0:S, 0, hh, 0:128],
                        wenc[(p, hh)][0:S, tsl],
                        start=True, stop=False, skip_group_check=True)
                else:
                    pi = stage - 1
                    nc.tensor.matmul(
                        pvt[tci],
                        vT[p][:, 2 * pi + 1:2 * pi + 3, hh, 0:128],
                        wdr[(p, hh, pi)][:, :, tsl],
                        start=False, stop=(pi == 3), perf_mode=DRMODE,
                        skip_group_check=True)

        def emit_normalize_hh(p, pvt, hh):
            recipb_h = hp.tile([64, T], F32, tag=f"recipb{hh}",
                               name=f"recipb_{p}_{hh}")
            for tci in range(2):
                # den rides psum row 0 (ones-column is vT col 0), so the
                # custom-DVE recip -- which always reads partition 0 -- can
                # take the PSUM AP directly; no ScalarE copy.
                rec_g = hp.tile([1, 512], F32, tag=f"rec{hh}{tci}",
                                name=f"rec_{p}_{hh}_{tci}")
                nc.vector.reciprocal_approx_fast(
                    out=rec_g, in_=pvt[tci][0:1, :])
                nc.gpsimd.partition_broadcast(
                    recipb_h[:, 512 * tci:512 * (tci + 1)], rec_g)
            if DEBUG_DUMPS and p == 0 and hh == 0:
                nc.sync.dma_start(out=dbg_rec_d.ap(), in_=rec_h)
                nc.sync.dma_start(out=dbg_rcb_d.ap(), in_=recipb_h)
                pvn = stats.tile([8, 2, 512], F32, name="pvn_sb")
                for tci in range(2):
                    nc.vector.tensor_copy(out=pvn[:, tci, :],
                                          in_=pvt[tci][0:8, :])
                nc.sync.dma_start(out=dbg_pvn_d.ap(), in_=pvn)
            rlo = 64 * hh
            for tci in range(2):
                nc.vector.tensor_mul(
                    out=att_all[rlo:rlo + 64, p, 512 * tci:512 * (tci + 1)],
                    in0=pvt[tci][64:128, :],
                    in1=recipb_h[:, 512 * tci:512 * (tci + 1)])

        # prologue: pair 0 q/k projections only (vT units interleave into
        # pair 0's own j-loop; PV needs them only at pair end)
        alloc_pair(0)
        for bi in range(2):
            for tci in range(2):
                emit_qk_unit(0, bi, tci)

        for p in range(4):
            units = [("vt", p, j) for j in range(1, NT)]
            if p < 3:
                alloc_pair(p + 1)
                units += [("qk", p + 1, bi, tci)
                          for bi in range(2) for tci in range(2)]
            for j in range(NT):
                emit_qk_exp(p, j)
                if units:
                    take = 1 if j < 4 else 2
                    for u in units[:take]:
                        emit_unit(u)
                    units = units[take:]
            for hh in range(2):
                pvt = [psum.tile([128, 512], F32, tag="pv", bufs=2,
                                 name=f"pv_{p}_{hh}_{tci}") for tci in range(2)]
                for stage in range(5):
                    emit_pv_stage(p, pvt, hh, stage)
                emit_normalize_hh(p, pvt, hh)
                if hh == 0:
                    for u in units:
                        emit_unit(u)
                    units = []
            if DEBUG_DUMPS and p == 0:
                nc.sync.dma_start(out=dbg_qq_d.ap(), in_=qq[0])
                nc.sync.dma_start(out=dbg_we_d.ap(), in_=wenc[(0, 0)])
                for pi in range(4):
                    nc.sync.dma_start(out=dbg_wd_d.ap()[:, pi, :, :],
                                      in_=wdr[(0, 0, pi)])

        if DEBUG_DUMPS:
            nc.sync.dma_start(out=dbg_nrm_d.ap(), in_=nrm)
            nc.sync.dma_start(out=dbg_kk_d.ap(), in_=kk[0])
            nc.sync.dma_start(out=dbg_vt_d.ap(), in_=vT[0])
            nc.sync.dma_start(out=dbg_att_d.ap(), in_=att_all)

        # ---- output projection + residual ----------------------------------
        # residual x is added into the proj psum via an identity matmul (PE
        # and ScalarE are idle at the tail; VectorE is still draining exps).
        opool = ctx.enter_context(tc.tile_pool(name="opool", bufs=2))
        for i in range(4):
            for tci in range(2):
                tsl = slice(512 * tci, 512 * (tci + 1))
                pr_ps = psum.tile([128, 512], F32, tag="u", bufs=2,
                                  name=f"pr_{i}_{tci}")
                nc.tensor.matmul(
                    pr_ps, identf, x_sb[:, i, tsl],
                    start=True, stop=False, skip_group_check=True)
                for kc in range(4):
                    nc.tensor.matmul(
                        pr_ps, wp[:, kc, 128 * i:128 * (i + 1)],
                        att_all[:, kc, tsl],
                        start=False, stop=(kc == 3), skip_group_check=True,
                    )
                out_sb = opool.tile([128, 512], F32, tag="osb")
                nc.scalar.activation(
                    out=out_sb, in_=pr_ps, func=AF.Identity,
                    bias=pb[:, i:i + 1], scale=1.0)
                eng = nc.sync if tci == 0 else nc.scalar
                eng.dma_start(out=out_d.ap()[:, i, tsl], in_=out_sb)

    nc.compile()
    return nc


def _to_part_major(a, inner):
    """[C, inner...] with C=512 -> [128, 4, inner] (c = 128*i + p)."""
    return np.ascontiguousarray(
        a.reshape(4, 128, inner).transpose(1, 0, 2))


def prep_inputs(x, encoder_out, capt_attn_mask, norm_scale, norm_bias,
                qkv_w, qkv_b, ekv_w, ekv_b, proj_w, proj_b):
    """Host-side marshalling: shard over batch + transpose/cast weights."""
    x = np.asarray(x, np.float32).reshape(B, C, T)
    enc = np.asarray(encoder_out, np.float32)
    mask = np.asarray(capt_attn_mask).astype(bool)

    x_dev = x.reshape(B, 4, 128, T).transpose(0, 2, 1, 3)
    enc_pad = np.zeros((B, C, 80), np.float32)
    enc_pad[:, :, 0:S] = enc
    enc_dev = enc_pad.reshape(B, 4, 128, 80).transpose(0, 2, 1, 3).astype(e4m3)

    # mask columns: col0 = m (0/1, rows>=77 -> 1), col1 = m/16
    mvec = np.ones((B, 128), np.float32)
    mvec[:, 0:S] = mask.astype(np.float32)
    msk = np.stack([mvec, mvec / WSCALE], axis=2)  # [B, 128, 2]

    # weight rows permuted into per-pair block layout:
    # [q_h|q_h1|k_h|k_h1|v_h|v_h1] per pair.
    qperm = np.array([
        192 * (2 * p + hh) + 64 * b + o
        for p in range(4) for b in range(3) for hh in range(2) for o in range(64)
    ])
    eperm = np.array([
        128 * (2 * p + hh) + 64 * b + o
        for p in range(4) for b in range(2) for hh in range(2) for o in range(64)
    ])
    wq_t = _to_part_major(
        np.asarray(qkv_w, np.float32)[qperm].T * WSCALE, 3 * C).astype(e4m3)
    we_t = _to_part_major(
        np.asarray(ekv_w, np.float32)[eperm].T * WSCALE, 2 * C).astype(e4m3)
    wp_t = _to_part_major(np.asarray(proj_w, np.float32).T, C).astype(bf16)

    qkv_b = np.asarray(qkv_b, np.float32)
    ekv_b = np.asarray(ekv_b, np.float32)
    # v / ev biases are folded nowhere -- must be zero (true for this problem)
    for p in range(4):
        for hh in range(2):
            h = 2 * p + hh
            assert np.all(qkv_b[192 * h + 128:192 * h + 192] == 0.0), "v bias != 0"
            assert np.all(ekv_b[128 * h + 64:128 * h + 128] == 0.0), "ev bias != 0"
    qbm = np.zeros((128, 4, 2), np.float32)
    ebm = np.zeros((128, 1), np.float32)
    for p in range(4):
        h = 2 * p
        for bi in range(2):
            qbm[0:64, p, bi] = qkv_b[192 * h + 64 * bi:192 * h + 64 * bi + 64]
            qbm[64:128, p, bi] = qkv_b[192 * (h + 1) + 64 * bi:192 * (h + 1) + 64 * bi + 64]
        ebm[0:64, 0] = ekv_b[128 * h:128 * h + 64]
        ebm[64:128, 0] = ekv_b[128 * (h + 1):128 * (h + 1) + 64]
    pbm = np.ascontiguousarray(np.asarray(proj_b, np.float32).reshape(4, 128).T)
    gamm = np.ascontiguousarray(np.asarray(norm_scale, np.float32).reshape(4, 128).T)
    betm = np.ascontiguousarray(np.asarray(norm_bias, np.float32).reshape(4, 128).T)

    selA = np.zeros((128, 8), np.float32)
    for pp in range(128):
        selA[pp, pp // 16] = 1.0
    selAT = np.ascontiguousarray(selA.T)

    shared = {"wq": wq_t, "we": we_t, "wp": wp_t, "qb": qbm, "eb": ebm,
              "pb": pbm, "gam": gamm, "bet": betm, "selA": selA,
              "selAT": selAT}
    in_maps = []
    for b in range(B):
        m = dict(shared)
        m["x"] = np.ascontiguousarray(x_dev[b])
        m["enc"] = np.ascontiguousarray(enc_dev[b])
        m["msk"] = np.ascontiguousarray(msk[b])
        in_maps.append(m)
    return in_maps


def gather_output(results):
    out = np.stack([r["out"] for r in results])  # [8, 128, 4, T]
    return np.ascontiguousarray(
        out.transpose(0, 2, 1, 3).reshape(B, C, HH, WW).astype(np.float32))


_NC = None


def _get_nc():
    global _NC
    if _NC is None:
        _NC = build_program()
    return _NC


def kernel(**inputs) -> np.ndarray:
    from concourse.bass_utils import run_bass_kernel_spmd

    nc = _get_nc()
    in_maps = prep_inputs(**inputs)
    res = run_bass_kernel_spmd(nc, in_maps, core_ids=list(range(N_CORES)))
    return gather_output(res.results)


if __name__ == "__main__":
    nc = build_program()
    print("program built ok")


# revision 52
# speedup vs baseline: 1.0793x; 1.0025x over previous
"""AttentionBlock (GroupNorm32 + self/cross attention + proj + residual) on 8 TRN2 cores.

Sharding: data-parallel over batch. B=8 samples, one per NeuronCore.

v2 design (vs baseline):
  - fp8(e4m3) DoubleRow matmuls for qkv/ekv projections and PV (pairs of
    s-tiles contract 256 rows per MM). Weights pre-scaled x16 on host to
    avoid fp8 subnormals; un-scaled in the PSUM evacuation.
  - v is produced already TRANSPOSED on the PE (vT = nrm^T @ WvT, N=128
    fp8 matmuls) -- no PE transposes, no v/ev evacuation.
  - s-axis tiles: [0:77) encoder tile + 8 aligned self tiles of 128.
  - caption mask folded into vT as data: ones-column = mask (0/1), masked
    ev rows zeroed. No logit bias anywhere; exact vs reference (-1e4 mask
    underflows exp to 0 in fp32 too).
  - exp split: ScalarE real exp -> fp8 for s-tiles 0..4; VectorE
    int8-Schraudolph bit-trick exp -> fp8 for s-tiles 5..8. All wgt tiles
    fp8; PV per (head, tci) = 1 plain MM (enc) + 4 DR MMs.
  - GroupNorm(32) group stats + per-channel broadcast via tiny fp32
    selector matmuls on the PE (no transposes, no DMA broadcast).
  - softmax denominators ride the vT ones-column (PV psum row 64);
    normalize = DVE reciprocal (PSUM read) -> gpsimd partition_broadcast
    -> one [64,1024] TT multiply per head.
  - output projection bf16 from att_all; residual+bias via one STT per
    block; per-block output DMA.
"""

import sys
from contextlib import ExitStack

import numpy as np

for _p in ("/opt/trn_rl_repo",):
    if _p not in sys.path:
        sys.path.insert(0, _p)

import ml_dtypes  # noqa: E402

import concourse.bass as bass  # noqa: E402
import concourse.tile as tile  # noqa: E402
from concourse import bacc, mybir  # noqa: E402
from concourse.masks import make_identity  # noqa: E402

F32 = mybir.dt.float32
BF16 = mybir.dt.bfloat16
FP8 = mybir.dt.float8e4
I8 = mybir.dt.int8
AF = mybir.ActivationFunctionType
ALU = mybir.AluOpType
DRMODE = mybir.MatmulPerfMode.DoubleRow

B, C, HH, WW = 8, 512, 32, 32
T = HH * WW          # 1024
HEADS, CH, S = 8, 64, 77
ST = S + T           # 1101
NT = 9               # s-tiles: [0:77) enc + 8 x 128 self
GCH = 16             # channels per GroupNorm group
N_CORES = 8
WSCALE = 16.0        # host-side weight pre-scale (fp8 subnormal dodge)

# Schraudolph fp8(e4m3) exp: y ~= bitcast8(int8(A8*x + B8))
A8 = 8.0 / float(np.log(2.0))    # 11.5416
B8 = 56.0 - 0.397                # bias 7*8 with mid-point correction

e4m3 = ml_dtypes.float8_e4m3
bf16 = ml_dtypes.bfloat16

DEBUG_DUMPS = False


def tile_rows(j):
    return S if j == 0 else 128


def tile_ssl(j):
    return slice(0, S) if j == 0 else slice(S + 128 * (j - 1), S + 128 * j)


def exp_on_act(j, hh):
    """Per (j, hh): head 0 on ScalarE (real exp), head 1 mostly on VectorE
    (int8-Schraudolph), so both heads' exps run in parallel per s-tile.
    j=0 (enc, both heads) and j=1 of head 1 on ScalarE for balance."""
    return hh == 0 or j <= 1


def build_program():
    nc = bacc.Bacc("TRN2", target_bir_lowering=False, debug=False)

    x_d = nc.dram_tensor("x", [128, 4, T], F32, kind="ExternalInput")
    enc_d = nc.dram_tensor("enc", [128, 4, 80], FP8, kind="ExternalInput")
    wq_d = nc.dram_tensor("wq", [128, 4, 3 * C], FP8, kind="ExternalInput")
    we_d = nc.dram_tensor("we", [128, 4, 2 * C], FP8, kind="ExternalInput")
    wp_d = nc.dram_tensor("wp", [128, 4, C], BF16, kind="ExternalInput")
    qb_d = nc.dram_tensor("qb", [128, 4, 2], F32, kind="ExternalInput")
    eb_d = nc.dram_tensor("eb", [128, 1], F32, kind="ExternalInput")
    pb_d = nc.dram_tensor("pb", [128, 4], F32, kind="ExternalInput")
    gam_d = nc.dram_tensor("gam", [128, 4], F32, kind="ExternalInput")
    bet_d = nc.dram_tensor("bet", [128, 4], F32, kind="ExternalInput")
    msk_d = nc.dram_tensor("msk", [128, 2], F32, kind="ExternalInput")
    selA_d = nc.dram_tensor("selA", [128, 8], F32, kind="ExternalInput")
    selAT_d = nc.dram_tensor("selAT", [8, 128], F32, kind="ExternalInput")
    out_d = nc.dram_tensor("out", [128, 4, T], F32, kind="ExternalOutput")
    if DEBUG_DUMPS:
        dbg_nrm_d = nc.dram_tensor("dbg_nrm", [128, 4, T], FP8, kind="ExternalOutput")
        dbg_qq_d = nc.dram_tensor("dbg_qq", [128, T], BF16, kind="ExternalOutput")
        dbg_kk_d = nc.dram_tensor("dbg_kk", [128, ST], BF16, kind="ExternalOutput")
        dbg_vt_d = nc.dram_tensor("dbg_vt", [128, NT, 2, 128], FP8, kind="ExternalOutput")
        dbg_we_d = nc.dram_tensor("dbg_we", [128, T], FP8, kind="ExternalOutput")
        dbg_wd_d = nc.dram_tensor("dbg_wd", [128, 4, 2, T], FP8, kind="ExternalOutput")
        dbg_att_d = nc.dram_tensor("dbg_att", [128, 4, T], BF16, kind="ExternalOutput")
        dbg_den_d = nc.dram_tensor("dbg_den", [1, 2, 2, 512], F32, kind="ExternalOutput")
        dbg_rec_d = nc.dram_tensor("dbg_rec", [1, 2, 512], F32, kind="ExternalOutput")
        dbg_rcb_d = nc.dram_tensor("dbg_rcb", [64, T], F32, kind="ExternalOutput")
        dbg_pvn_d = nc.dram_tensor("dbg_pvn", [8, 2, 512], F32, kind="ExternalOutput")

    with tile.TileContext(nc) as tc, ExitStack() as ctx:
        consts = ctx.enter_context(tc.tile_pool(name="consts", bufs=1))
        stats = ctx.enter_context(tc.tile_pool(name="stats", bufs=1))
        kkp = ctx.enter_context(tc.tile_pool(name="kkp", bufs=1))
        hp = ctx.enter_context(tc.tile_pool(name="hp", bufs=2))
        wgtp = ctx.enter_context(tc.tile_pool(name="wgtp", bufs=2))
        psum = ctx.enter_context(tc.tile_pool(name="psum", bufs=2, space="PSUM"))

        # ---- constant loads (small first; x in 4 chunks) -------------------
        enc_sb = consts.tile([128, 4, 80], FP8)
        nc.sync.dma_start(out=enc_sb, in_=enc_d.ap())
        we = consts.tile([128, 4, 2 * C], FP8)
        nc.sync.dma_start(out=we, in_=we_d.ap())
        msk = consts.tile([128, 2], F32)
        nc.sync.dma_start(out=msk, in_=msk_d.ap())
        selA = consts.tile([128, 8], F32)
        nc.sync.dma_start(out=selA, in_=selA_d.ap())
        selAT = consts.tile([8, 128], F32)
        nc.sync.dma_start(out=selAT, in_=selAT_d.ap())
        qb = consts.tile([128, 4, 2], F32)
        nc.sync.dma_start(out=qb, in_=qb_d.ap())
        eb = consts.tile([128, 1], F32)
        nc.sync.dma_start(out=eb, in_=eb_d.ap())
        pb = consts.tile([128, 4], F32)
        nc.sync.dma_start(out=pb, in_=pb_d.ap())
        gam = consts.tile([128, 4], F32)
        nc.sync.dma_start(out=gam, in_=gam_d.ap())
        bet = consts.tile([128, 4], F32)
        nc.sync.dma_start(out=bet, in_=bet_d.ap())
        x_sb = consts.tile([128, 4, T], F32)
        for i in range(4):
            for hf in range(2):
                eng = nc.sync if (2 * i + hf) % 2 == 0 else nc.scalar
                eng.dma_start(
                    out=x_sb[:, i, 512 * hf:512 * (hf + 1)],
                    in_=x_d.ap()[:, i, 512 * hf:512 * (hf + 1)])
        wq = consts.tile([128, 4, 3 * C], FP8)
        nc.scalar.dma_start(out=wq, in_=wq_d.ap())
        wp = consts.tile([128, 4, C], BF16)
        nc.sync.dma_start(out=wp, in_=wp_d.ap())
        identf = consts.tile([128, 128], F32)
        make_identity(nc, identf)

        # ---- per-pair tensors ----------------------------------------------
        # kk/vT for all 4 pairs at once (ek/evT can run before x arrives).
        kk = [kkp.tile([128, ST], BF16, name=f"kk_{p}") for p in range(4)]
        vT = [kkp.tile([128, NT, 2, 128], FP8, name=f"vT_{p}") for p in range(4)]
        qq = {}
        wenc = {}
        wdr = {}

        def alloc_pair(p):
            qq[p] = hp.tile([128, T], BF16, tag="qq", name=f"qq_{p}")
            for hh in range(2):
                wenc[(p, hh)] = wgtp.tile(
                    [128, T], FP8, tag=f"wenc{hh}", name=f"wenc_{p}_{hh}")
                for pi in range(4):
                    wdr[(p, hh, pi)] = wgtp.tile(
                        [128, 2, T], FP8, tag=f"wdr{hh}{pi}",
                        name=f"wdr_{p}_{hh}_{pi}")

        # ---- early: ek + evT for all pairs (needs only enc/we) ------------
        # ones/mask columns of vT (col 64 of each hh block)
        for p in range(4):
            nc.gpsimd.memset(vT[p][:, :, :, 0:1], 1.0)
            for hh in range(2):
                nc.vector.tensor_copy(
                    out=vT[p][0:S, 0, hh, 0:1], in_=msk[0:S, 0:1])

        for p in range(4):
            # ek: kk[:, 0:77] = (WekT.T @ enc)/16 + ebias
            ek_ps = psum.tile([128, 80], F32, tag="u", bufs=2, name=f"ek_{p}")
            for kp in range(2):
                nc.tensor.matmul(
                    ek_ps,
                    we[:, 2 * kp:2 * kp + 2, 256 * p:256 * p + 128],
                    enc_sb[:, 2 * kp:2 * kp + 2, :],
                    start=(kp == 0), stop=(kp == 1), perf_mode=DRMODE)
            nc.scalar.activation(
                out=kk[p][:, 0:S], in_=ek_ps[:, 0:S], func=AF.Identity,
                bias=eb, scale=1.0 / WSCALE)
            # evT: vT[0:77, 0, :, 0:64] = (enc^T @ WevT) * mask/16
            ev_ps = psum.tile([80, 128], F32, tag="u", bufs=2, name=f"ev_{p}")
            for kp in range(2):
                nc.tensor.matmul(
                    ev_ps[0:S, :],
                    enc_sb[:, 2 * kp:2 * kp + 2, 0:S],
                    we[:, 2 * kp:2 * kp + 2, 256 * p + 128:256 * p + 256],
                    start=(kp == 0), stop=(kp == 1), perf_mode=DRMODE)
            nc.scalar.activation(
                out=vT[p][0:S, 0, :, 64:128],
                in_=ev_ps[0:S, :].rearrange("p (h c) -> p h c", c=64),
                func=AF.Copy, scale=msk[0:S, 1:2])

        # ---- GroupNorm(32) via selector matmuls ----------------------------
        mv = stats.tile([128, 4, 2], F32)
        for i in range(4):
            bnst = stats.tile([128, 2, 6], F32, tag="bnst", bufs=2)
            nc.vector.bn_stats(out=bnst[:, 0, :], in_=x_sb[:, i, 0:512])
            nc.vector.bn_stats(out=bnst[:, 1, :], in_=x_sb[:, i, 512:1024])
            nc.vector.bn_aggr(out=mv[:, i, :], in_=bnst)

        stmx = stats.tile([128, 8], F32)
        nc.vector.tensor_copy(out=stmx[:, 0:4], in_=mv[:, :, 0])
        nc.vector.tensor_mul(out=stmx[:, 4:8], in0=mv[:, :, 0], in1=mv[:, :, 0])
        nc.vector.tensor_add(out=stmx[:, 4:8], in0=stmx[:, 4:8], in1=mv[:, :, 1])

        gps = psum.tile([8, 8], F32, tag="u", bufs=2, name="gps")
        nc.tensor.matmul(gps, selA, stmx, start=True, stop=True)
        mug = stats.tile([8, 4], F32)
        nc.vector.tensor_scalar_mul(out=mug, in0=gps[:, 0:4], scalar1=1.0 / GCH)
        varg = stats.tile([8, 4], F32)
        nc.vector.tensor_scalar_mul(out=varg, in0=gps[:, 4:8], scalar1=1.0 / GCH)
        tmp = stats.tile([8, 4], F32)
        nc.vector.tensor_mul(out=tmp, in0=mug, in1=mug)
        nc.vector.tensor_sub(out=varg, in0=varg, in1=tmp)
        eps_t = stats.tile([8, 1], F32)
        nc.vector.memset(eps_t, 1e-5)
        lnv = stats.tile([8, 4], F32)
        nc.scalar.activation(out=lnv, in_=varg, func=AF.Ln, bias=eps_t, scale=1.0)
        mr = stats.tile([8, 8], F32)
        nc.vector.tensor_copy(out=mr[:, 0:4], in_=mug)
        nc.scalar.activation(out=mr[:, 4:8], in_=lnv, func=AF.Exp, scale=-0.5)

        cols = psum.tile([128, 8], F32, tag="u", bufs=2, name="cols")
        nc.tensor.matmul(cols, selAT, mr, start=True, stop=True)
        a_col = stats.tile([128, 4], F32)
        nc.vector.tensor_mul(out=a_col, in0=cols[:, 4:8], in1=gam)
        b_col = stats.tile([128, 4], F32)
        nc.vector.tensor_mul(out=b_col, in0=cols[:, 0:4], in1=a_col)
        nc.vector.tensor_sub(out=b_col, in0=bet, in1=b_col)

        nrm = consts.tile([128, 4, T], FP8)
        for i in range(4):
            nc.vector.tensor_scalar(
                out=nrm[:, i, :], in0=x_sb[:, i, :],
                scalar1=a_col[:, i:i + 1], scalar2=b_col[:, i:i + 1],
                op0=ALU.mult, op1=ALU.add)

        att_all = consts.tile([128, 4, T], BF16)

        # ---- per-pair projection units -------------------------------------
        def emit_qk_unit(p, bi, tci):  # noqa: renamed args used via emit_unit
            """q (bi=0) / k (bi=1) projection, t-half tci: 2 DR matmuls."""
            tsl = slice(512 * tci, 512 * (tci + 1))
            mm_ps = psum.tile([128, 512], F32, tag="u", bufs=2,
                              name=f"mm_{p}_{bi}_{tci}")
            cl = 384 * p + 128 * bi
            for kp in range(2):
                nc.tensor.matmul(
                    mm_ps,
                    wq[:, 2 * kp:2 * kp + 2, cl:cl + 128],
                    nrm[:, 2 * kp:2 * kp + 2, tsl],
                    start=(kp == 0), stop=(kp == 1), perf_mode=DRMODE)
            dest = qq[p][:, tsl] if bi == 0 else kk[p][:, S + 512 * tci:S + 512 * (tci + 1)]
            if bi == 1 and tci == 1:
                nc.vector.tensor_scalar(
                    out=dest, in0=mm_ps, scalar1=1.0 / WSCALE,
                    scalar2=qb[:, p, bi:bi + 1], op0=ALU.mult, op1=ALU.add)
            else:
                nc.scalar.activation(
                    out=dest, in_=mm_ps, func=AF.Identity,
                    bias=qb[:, p, bi:bi + 1], scale=1.0 / WSCALE)

        def emit_vt_unit(p, j):  # noqa
            """self vT tile j (1..8): vT[:, j, :, 0:64] = nrm^T @ WvT / 16."""
            tsl = slice(128 * (j - 1), 128 * j)
            vt_ps = psum.tile([128, 128], F32, tag="u", bufs=2,
                              name=f"vt_{p}_{j}")
            cl = 384 * p + 256
            for kc in range(4):
                nc.tensor.matmul(
                    vt_ps, nrm[:, kc, tsl], wq[:, kc, cl:cl + 128],
                    start=(kc == 0), stop=(kc == 3))
            nc.vector.tensor_scalar(
                out=vT[p][:, j, :, 64:128],
                in0=vt_ps.rearrange("p (h c) -> p h c", c=64),
                scalar1=1.0 / WSCALE, scalar2=0.0,
                op0=ALU.mult, op1=ALU.add)

        def emit_unit(u):
            if u[0] == "qk":
                emit_qk_unit(u[1], u[2], u[3])
            else:
                emit_vt_unit(u[1], u[2])

        # ---- attention -----------------------------------------------------
        def emit_qk_exp(p, j):
            rows = tile_rows(j)
            ssl = tile_ssl(j)
            # per-(hh, tci) [128, 512] psum tiles: fine-grained ring so the
            # exp of tile (j, h0, t0) overlaps QK of (j, ., t1) and frees its
            # slot early for j+1.
            qk = {}
            for tci in range(2):
                for hh in range(2):
                    qk[(hh, tci)] = psum.tile(
                        [128, 512], F32, tag="qk", bufs=4,
                        name=f"qk_{p}_{j}_{hh}_{tci}")
                    rlo = 64 * hh
                    nc.tensor.matmul(
                        qk[(hh, tci)][0:rows, :],
                        kk[p][rlo:rlo + 64, ssl],
                        qq[p][rlo:rlo + 64, 512 * tci:512 * (tci + 1)],
                    )
                # emit exps for this tci right away (tci-granular drain)
                for hh in range(2):
                    if j == 0:
                        dsl = wenc[(p, hh)][0:rows, 512 * tci:512 * (tci + 1)]
                        dsl8 = None
                    else:
                        pi = (j - 1) // 2
                        sl = (j - 1) % 2
                        dsl = wdr[(p, hh, pi)][0:rows, sl,
                                               512 * tci:512 * (tci + 1)]
                        dsl8 = wdr[(p, hh, pi)].bitcast(I8)[
                            0:rows, sl, 512 * tci:512 * (tci + 1)]
                    if exp_on_act(j, hh):
                        nc.scalar.activation(
                            out=dsl, in_=qk[(hh, tci)][0:rows, :],
                            func=AF.Exp, scale=0.125)
                    else:
                        nc.vector.tensor_scalar(
                            out=dsl8, in0=qk[(hh, tci)][0:rows, :],
                            scalar1=A8 * 0.125, scalar2=B8,
                            op0=ALU.mult, op1=ALU.add)

        def emit_pv_stage(p, pvt, hh, stage):
            """stage 0: encoder tile (start); 1..4: DR pair pi=stage-1 (stop
            on 4). tci-inner so matmuls sharing a stationary operand are
            adjacent."""
            for tci in range(2):
                tsl = slice(512 * tci, 512 * (tci + 1))
                if stage == 0:
                    nc.tensor.matmul(
                        pvt[tci],
                        vT[p][0:S, 0, hh, 0:128],
                        wenc[(p, hh)][0:S, tsl],
                        start=True, stop=False, skip_group_check=True)
                else:
                    pi = stage - 1
                    nc.tensor.matmul(
                        pvt[tci],
                        vT[p][:, 2 * pi + 1:2 * pi + 3, hh, 0:128],
                        wdr[(p, hh, pi)][:, :, tsl],
                        start=False, stop=(pi == 3), perf_mode=DRMODE,
                        skip_group_check=True)

        def emit_normalize_hh(p, pvt, hh):
            recipb_h = hp.tile([64, T], F32, tag=f"recipb{hh}",
                               name=f"recipb_{p}_{hh}")
            for tci in range(2):
                # den rides psum row 0 (ones-column is vT col 0), so the
                # custom-DVE recip -- which always reads partition 0 -- can
                # take the PSUM AP directly; no ScalarE copy.
                rec_g = hp.tile([1, 512], F32, tag=f"rec{hh}{tci}",
                                name=f"rec_{p}_{hh}_{tci}")
                nc.vector.reciprocal_approx_fast(
                    out=rec_g, in_=pvt[tci][0:1, :])
                nc.gpsimd.partition_broadcast(
                    recipb_h[:, 512 * tci:512 * (tci + 1)], rec_g)
            if DEBUG_DUMPS and p == 0 and hh == 0:
                nc.sync.dma_start(out=dbg_rec_d.ap(), in_=rec_h)
                nc.sync.dma_start(out=dbg_rcb_d.ap(), in_=recipb_h)
                pvn = stats.tile([8, 2, 512], F32, name="pvn_sb")
                for tci in range(2):
                    nc.vector.tensor_copy(out=pvn[:, tci, :],
                                          in_=pvt[tci][0:8, :])
                nc.sync.dma_start(out=dbg_pvn_d.ap(), in_=pvn)
            rlo = 64 * hh
            for tci in range(2):
                nc.vector.tensor_mul(
                    out=att_all[rlo:rlo + 64, p, 512 * tci:512 * (tci + 1)],
                    in0=pvt[tci][64:128, :],
                    in1=recipb_h[:, 512 * tci:512 * (tci + 1)])

        # prologue: pair 0 q/k projections only (vT units interleave into
        # pair 0's own j-loop; PV needs them only at pair end)
        alloc_pair(0)
        for bi in range(2):
            for tci in range(2):
                emit_qk_unit(0, bi, tci)

        for p in range(4):
            units = [("vt", p, j) for j in range(1, NT)]
            if p < 3:
                alloc_pair(p + 1)
                units += [("qk", p + 1, bi, tci)
                          for bi in range(2) for tci in range(2)]
            for j in range(NT):
                emit_qk_exp(p, j)
                if units:
                    take = 1 if j < 4 else 2
                    for u in units[:take]:
                        emit_unit(u)
                    units = units[take:]
            for hh in range(2):
                pvt = [psum.tile([128, 512], F32, tag="pv", bufs=2,
                                 name=f"pv_{p}_{hh}_{tci}") for tci in range(2)]
                for stage in range(5):
                    emit_pv_stage(p, pvt, hh, stage)
                emit_normalize_hh(p, pvt, hh)
                if hh == 0:
                    for u in units:
                        emit_unit(u)
                    units = []
            if DEBUG_DUMPS and p == 0:
                nc.sync.dma_start(out=dbg_qq_d.ap(), in_=qq[0])
                nc.sync.dma_start(out=dbg_we_d.ap(), in_=wenc[(0, 0)])
                for pi in range(4):
                    nc.sync.dma_start(out=dbg_wd_d.ap()[:, pi, :, :],
                                      in_=wdr[(0, 0, pi)])

        if DEBUG_DUMPS:
            nc.sync.dma_start(out=dbg_nrm_d.ap(), in_=nrm)
            nc.sync.dma_start(out=dbg_kk_d.ap(), in_=kk[0])
            nc.sync.dma_start(out=dbg_vt_d.ap(), in_=vT[0])
            nc.sync.dma_start(out=dbg_att_d.ap(), in_=att_all)

        # ---- output projection + residual ----------------------------------
        # residual x is added into the proj psum via an identity matmul (PE
        # and ScalarE are idle at the tail; VectorE is still draining exps).
        opool = ctx.enter_context(tc.tile_pool(name="opool", bufs=2))
        for i in range(4):
            for tci in range(2):
                tsl = slice(512 * tci, 512 * (tci + 1))
                pr_ps = psum.tile([128, 512], F32, tag="u", bufs=2,
                                  name=f"pr_{i}_{tci}")
                nc.tensor.matmul(
                    pr_ps, identf, x_sb[:, i, tsl],
                    start=True, stop=False, skip_group_check=True)
                for kc in range(4):
                    nc.tensor.matmul(
                        pr_ps, wp[:, kc, 128 * i:128 * (i + 1)],
                        att_all[:, kc, tsl],
                        start=False, stop=(kc == 3), skip_group_check=True,
                    )
                out_sb = opool.tile([128, 512], F32, tag="osb")
                nc.scalar.activation(
                    out=out_sb, in_=pr_ps, func=AF.Identity,
                    bias=pb[:, i:i + 1], scale=1.0)
                nc.sync.dma_start(out=out_d.ap()[:, i, tsl], in_=out_sb)

    nc.compile()
    return nc


def _to_part_major(a, inner):
    """[C, inner...] with C=512 -> [128, 4, inner] (c = 128*i + p)."""
    return np.ascontiguousarray(
        a.reshape(4, 128, inner).transpose(1, 0, 2))


def prep_inputs(x, encoder_out, capt_attn_mask, norm_scale, norm_bias,
                qkv_w, qkv_b, ekv_w, ekv_b, proj_w, proj_b):
    """Host-side marshalling: shard over batch + transpose/cast weights."""
    x = np.asarray(x, np.float32).reshape(B, C, T)
    enc = np.asarray(encoder_out, np.float32)
    mask = np.asarray(capt_attn_mask).astype(bool)

    x_dev = x.reshape(B, 4, 128, T).transpose(0, 2, 1, 3)
    enc_pad = np.zeros((B, C, 80), np.float32)
    enc_pad[:, :, 0:S] = enc
    enc_dev = enc_pad.reshape(B, 4, 128, 80).transpose(0, 2, 1, 3).astype(e4m3)

    # mask columns: col0 = m (0/1, rows>=77 -> 1), col1 = m/16
    mvec = np.ones((B, 128), np.float32)
    mvec[:, 0:S] = mask.astype(np.float32)
    msk = np.stack([mvec, mvec / WSCALE], axis=2)  # [B, 128, 2]

    # weight rows permuted into per-pair block layout:
    # [q_h|q_h1|k_h|k_h1|v_h|v_h1] per pair.
    qperm = np.array([
        192 * (2 * p + hh) + 64 * b + o
        for p in range(4) for b in range(3) for hh in range(2) for o in range(64)
    ])
    eperm = np.array([
        128 * (2 * p + hh) + 64 * b + o
        for p in range(4) for b in range(2) for hh in range(2) for o in range(64)
    ])
    wq_t = _to_part_major(
        np.asarray(qkv_w, np.float32)[qperm].T * WSCALE, 3 * C).astype(e4m3)
    we_t = _to_part_major(
        np.asarray(ekv_w, np.float32)[eperm].T * WSCALE, 2 * C).astype(e4m3)
    wp_t = _to_part_major(np.asarray(proj_w, np.float32).T, C).astype(bf16)

    qkv_b = np.asarray(qkv_b, np.float32)
    ekv_b = np.asarray(ekv_b, np.float32)
    # v / ev biases are folded nowhere -- must be zero (true for this problem)
    for p in range(4):
        for hh in range(2):
            h = 2 * p + hh
            assert np.all(qkv_b[192 * h + 128:192 * h + 192] == 0.0), "v bias != 0"
            assert np.all(ekv_b[128 * h + 64:128 * h + 128] == 0.0), "ev bias != 0"
    qbm = np.zeros((128, 4, 2), np.float32)
    ebm = np.zeros((128, 1), np.float32)
    for p in range(4):
        h = 2 * p
        for bi in range(2):
            qbm[0:64, p, bi] = qkv_b[192 * h + 64 * bi:192 * h + 64 * bi + 64]
            qbm[64:128, p, bi] = qkv_b[192 * (h + 1) + 64 * bi:192 * (h + 1) + 64 * bi + 64]
        ebm[0:64, 0] = ekv_b[128 * h:128 * h + 64]
        ebm[64:128, 0] = ekv_b[128 * (h + 1):128 * (h + 1) + 64]
    pbm = np.ascontiguousarray(np.asarray(proj_b, np.float32).reshape(4, 128).T)
    gamm = np.ascontiguousarray(np.asarray(norm_scale, np.float32).reshape(4, 128).T)
    betm = np.ascontiguousarray(np.asarray(norm_bias, np.float32).reshape(4, 128).T)

    selA = np.zeros((128, 8), np.float32)
    for pp in range(128):
        selA[pp, pp // 16] = 1.0
    selAT = np.ascontiguousarray(selA.T)

    shared = {"wq": wq_t, "we": we_t, "wp": wp_t, "qb": qbm, "eb": ebm,
              "pb": pbm, "gam": gamm, "bet": betm, "selA": selA,
              "selAT": selAT}
    in_maps = []
    for b in range(B):
        m = dict(shared)
        m["x"] = np.ascontiguousarray(x_dev[b])
        m["enc"] = np.ascontiguousarray(enc_dev[b])
        m["msk"] = np.ascontiguousarray(msk[b])
        in_maps.append(m)
    return in_maps


def gather_output(results):
    out = np.stack([r["out"] for r in results])  # [8, 128, 4, T]
    return np.ascontiguousarray(
        out.transpose(0, 2, 1, 3).reshape(B, C, HH, WW).astype(np.float32))


_NC = None


def _get_nc():
    global _NC
    if _NC is None:
        _NC = build_program()
    return _NC


def kernel(**inputs) -> np.ndarray:
    from concourse.bass_utils import run_bass_kernel_spmd

    nc = _get_nc()
    in_maps = prep_inputs(**inputs)
    res = run_bass_kernel_spmd(nc, in_maps, core_ids=list(range(N_CORES)))
    return gather_output(res.results)


if __name__ == "__main__":
    nc = build_program()
    print("program built ok")
